# revision 1
# baseline (speedup 1.0000x reference)
"""Cross-attention Trainium2 kernel (nn_CrossAttention, B=2, L=2048, D=1024,
Dctx=768, 16 heads x 64).

Sharding: 8 cores = 2 (batch) x 4 (head-groups of 4 heads). Each core computes
its batch's Q/K/V projections for its 4 heads, flash-style attention in the
transposed (S^T) domain, and a partial output projection; the host sums the
head-group partials and adds b_o.

All activations live transposed on-chip (xT, ctxT, qT, kT, attnT) so every
matmul contracts over the partition dim with no on-chip transposes. The host
ships x/context pre-transposed. Matmuls run in float32r (fp32 rounded to
11-bit mantissa by the PE, full streaming rate). The softmax denominator is
produced by 32 ones-columns appended per head to the V weights, giving a
replicated d-block in PSUM that feeds a fast-reciprocal normalize on DVE.
"""
import numpy as np

import concourse.bass as bass
import concourse.tile as tile
from concourse import bacc, mybir, bass_utils

F32R = mybir.dt.float32r
F32 = mybir.dt.float32
EXP = mybir.ActivationFunctionType.Exp

# Problem shape (hardcoded per harness contract)
B, LQ, D = 2, 2048, 1024
DCTX = 768
NH, HD = 16, 64
SCALE = 1.0 / 8.0  # 1/sqrt(64)

# Per-core shard: 4 heads (one group), one batch
GH = 4                # heads per core
ONES = 32             # d-replication rows per head
VW = HD + ONES        # 96: per-head width in augmented V
VAW = GH * VW         # 384
KT_Q = D // 128       # 8
KT_C = DCTX // 128    # 6
NLK = LQ // 128       # 16 key tiles
NS = LQ // 512        # 4 query 512-slices
HALF = 1024


def _build():
    nc = bacc.Bacc("TRN2", target_bir_lowering=False, debug=False,
                   enable_asserts=False, num_devices=8)

    xT_d = nc.dram_tensor("xT", (D, LQ), F32R, kind="ExternalInput").ap()
    cT_d = nc.dram_tensor("ctxT", (DCTX, LQ), F32R, kind="ExternalInput").ap()
    wq_d = nc.dram_tensor("wq", (D, 256), F32R, kind="ExternalInput").ap()
    wk_d = nc.dram_tensor("wk", (DCTX, 256), F32R, kind="ExternalInput").ap()
    wv_d = nc.dram_tensor("wv", (DCTX, VAW), F32R, kind="ExternalInput").ap()
    wo_d = nc.dram_tensor("wo", (256, D), F32R, kind="ExternalInput").ap()
    bq_d = nc.dram_tensor("bq", (128, 2), F32, kind="ExternalInput").ap()
    bk_d = nc.dram_tensor("bk", (128, 2), F32, kind="ExternalInput").ap()
    bvb_d = nc.dram_tensor("bvb", (128, VAW), F32, kind="ExternalInput").ap()
    out_d = nc.dram_tensor("outT", (D, LQ), F32, kind="ExternalOutput").ap()
    import os
    dbg = os.environ.get("KDBG") == "1"
    if dbg:
        dpa_d = nc.dram_tensor("dbg_pa", (96, HALF), F32, kind="ExternalOutput").ap()
        drd_d = nc.dram_tensor("dbg_rd", (ONES, HALF), F32, kind="ExternalOutput").ap()
        dst_d = nc.dram_tensor("dbg_st", (128, HALF), F32, kind="ExternalOutput").ap()
        dex_d = nc.dram_tensor("dbg_ex", (128, HALF), F32, kind="ExternalOutput").ap()
        dv_d = nc.dram_tensor("dbg_v", (128, VAW), F32, kind="ExternalOutput").ap()

    with tile.TileContext(nc) as tc:
        with tc.tile_pool(name="w", bufs=1) as wp, \
             tc.tile_pool(name="xt", bufs=10) as xtp, \
             tc.tile_pool(name="ct", bufs=24) as ctp, \
             tc.tile_pool(name="act", bufs=1) as actp, \
             tc.tile_pool(name="expp", bufs=3) as expp, \
             tc.tile_pool(name="rdp", bufs=1) as rdp, \
             tc.tile_pool(name="outp", bufs=3) as outp, \
             tc.tile_pool(name="ps_mm", bufs=2, space="PSUM") as ps_mm, \
             tc.tile_pool(name="ps_s", bufs=2, space="PSUM") as ps_s, \
             tc.tile_pool(name="ps_at", bufs=1, space="PSUM") as ps_at:

            # ---- weights / biases ----
            wq_t = wp.tile([128, KT_Q * 256], F32R, tag="wq")
            nc.sync.dma_start(wq_t[:].rearrange("p (kt m) -> p kt m", m=256),
                              wq_d.rearrange("(kt p) m -> p kt m", p=128))
            wk_t = wp.tile([128, KT_C * 256], F32R, tag="wk")
            nc.sync.dma_start(wk_t[:].rearrange("p (kt m) -> p kt m", m=256),
                              wk_d.rearrange("(kt p) m -> p kt m", p=128))
            wv_t = wp.tile([128, KT_C * VAW], F32R, tag="wv")
            nc.sync.dma_start(wv_t[:].rearrange("p (kt m) -> p kt m", m=384),
                              wv_d.rearrange("(kt p) m -> p kt m", p=128))
            wo_t = wp.tile([128, 2 * D], F32R, tag="wo")
            nc.sync.dma_start(wo_t[:].rearrange("p (p2 m) -> p p2 m", m=1024),
                              wo_d.rearrange("(p2 p) m -> p p2 m", p=128))
            bq_t = wp.tile([128, 2], F32, tag="bq")
            nc.sync.dma_start(bq_t[:], bq_d[:])
            bk_t = wp.tile([128, 2], F32, tag="bk")
            nc.sync.dma_start(bk_t[:], bk_d[:])
            bvb_t = wp.tile([128, VAW], F32, tag="bvb")
            nc.sync.dma_start(bvb_t[:], bvb_d[:])

            # ---- persistent activation tiles ----
            qT = [actp.tile([128, LQ], F32R, tag=f"qT{p}", name=f"qT{p}")
                  for p in range(2)]
            kT = [actp.tile([128, LQ], F32R, tag=f"kT{p}", name=f"kT{p}")
                  for p in range(2)]
            v_t = actp.tile([128, NLK * VAW], F32R, tag="v")
            aT = [actp.tile([128, LQ], F32R, tag=f"aT{p}", name=f"aT{p}")
                  for p in range(2)]

            # ---- K projection (both pairs) + V projection, per 512-slice ----
            ct_tiles = {}
            for s in range(NS):
                for kt in range(KT_C):
                    t = ctp.tile([128, 512], F32R, tag="ct")
                    nc.sync.dma_start(
                        t[:], cT_d[128 * kt:128 * (kt + 1), 512 * s:512 * (s + 1)])
                    ct_tiles[(kt, s)] = t
                for p in range(2):
                    ps = ps_mm.tile([128, 512], F32, tag="mm")
                    for kt in range(KT_C):
                        nc.tensor.matmul(
                            ps[:], wk_t[:, 256 * kt + 128 * p:256 * kt + 128 * (p + 1)],
                            ct_tiles[(kt, s)][:],
                            start=(kt == 0), stop=(kt == KT_C - 1))
                    nc.vector.tensor_scalar_add(
                        kT[p][:, 512 * s:512 * (s + 1)], ps[:], bk_t[:, p:p + 1])

            # ---- Q projection (both pairs), per 512-slice ----
            for s in range(NS):
                xt_tiles = []
                for kt in range(KT_Q):
                    t = xtp.tile([128, 512], F32R, tag="xt")
                    nc.sync.dma_start(
                        t[:], xT_d[128 * kt:128 * (kt + 1), 512 * s:512 * (s + 1)])
                    xt_tiles.append(t)
                for p in range(2):
                    ps = ps_mm.tile([128, 512], F32, tag="mm")
                    for kt in range(KT_Q):
                        nc.tensor.matmul(
                            ps[:], wq_t[:, 256 * kt + 128 * p:256 * kt + 128 * (p + 1)],
                            xt_tiles[kt][:],
                            start=(kt == 0), stop=(kt == KT_Q - 1))
                    nc.vector.tensor_scalar_add(
                        qT[p][:, 512 * s:512 * (s + 1)], ps[:], bq_t[:, p:p + 1])

            def v_chunk(j):
                ps = ps_mm.tile([128, VAW], F32, tag="mm")
                s, jj = j // 4, j % 4
                for kt in range(KT_C):
                    nc.tensor.matmul(
                        ps[:],
                        ct_tiles[(kt, s)][:, 128 * jj:128 * (jj + 1)],
                        wv_t[:, VAW * kt:VAW * (kt + 1)],
                        start=(kt == 0), stop=(kt == KT_C - 1))
                nc.vector.tensor_add(v_t[:, VAW * j:VAW * (j + 1)], ps[:], bvb_t[:])

            # ---- attention per head; V chunks computed inline on first head ----
            for h in range(GH):
                p, m = h // 2, h % 2
                r0 = 64 * m
                for half in range(2):
                    pa = ps_at.tile([96, HALF], F32, tag="attn")
                    for j in range(NLK):
                        if h == 0 and half == 0:
                            v_chunk(j)
                        st = ps_s.tile([128, HALF], F32, tag="s")
                        for n in range(2):
                            nc.tensor.matmul(
                                st[:, 512 * n:512 * (n + 1)],
                                kT[p][r0:r0 + 64, 128 * j:128 * (j + 1)],
                                qT[p][r0:r0 + 64,
                                      HALF * half + 512 * n:HALF * half + 512 * (n + 1)],
                                start=True, stop=True)
                        ex = expp.tile([128, HALF], F32R, tag="expS")
                        if dbg and h == 0 and half == 0 and j == 0:
                            dt0 = expp.tile([128, HALF], F32, tag="expS")
                            nc.vector.tensor_copy(dt0[:], st[:])
                            nc.sync.dma_start(dst_d[:], dt0[:])
                        nc.scalar.activation(ex[:], st[:], EXP, scale=SCALE)
                        if dbg and h == 0 and half == 0 and j == 0:
                            nc.sync.dma_start(dex_d[:], ex[:].bitcast(F32))
                            nc.sync.dma_start(dv_d[:], v_t[:, 0:VAW].bitcast(F32))
                        for n in range(2):
                            nc.tensor.matmul(
                                pa[:, 512 * n:512 * (n + 1)],
                                v_t[:, VAW * j + VW * h:VAW * j + VW * h + VW],
                                ex[:, 512 * n:512 * (n + 1)],
                                start=(j == 0), stop=(j == NLK - 1))
                    # normalize: attnT = attnU * (1/d)
                    rd = rdp.tile([ONES, HALF], F32, tag="rd")
                    rds = rdp.tile([ONES, HALF], F32, tag="rds")
                    dsb = rdp.tile([ONES, HALF], F32, tag="dsb")
                    nc.vector.tensor_copy(dsb[:], pa[64:96, :])
                    nc.vector.reciprocal_approx_accurate(rd[:], dsb[:], rds[:])
                    if dbg and h == 0 and half == 0:
                        dt_ = expp.tile([96, HALF], F32, tag="expS")
                        nc.vector.tensor_copy(dt_[:], pa[:])
                        nc.sync.dma_start(dpa_d[:], dt_[:])
                        nc.sync.dma_start(drd_d[:], rd[:])
                    cols = slice(HALF * half, HALF * (half + 1))
                    nc.vector.tensor_mul(aT[p][r0:r0 + 32, cols], pa[0:32, :], rd[:])
                    nc.vector.tensor_mul(aT[p][r0 + 32:r0 + 64, cols], pa[32:64, :], rd[:])

            # ---- output projection: outT[m-slice, s-slice] ----
            for mo in range(D // 128):
                for s in range(NS):
                    ps = ps_mm.tile([128, 512], F32, tag="mm")
                    for p in range(2):
                        nc.tensor.matmul(
                            ps[:], wo_t[:, D * p + 128 * mo:D * p + 128 * (mo + 1)],
                            aT[p][:, 512 * s:512 * (s + 1)],
                            start=(p == 0), stop=(p == 1))
                    ot = outp.tile([128, 512], F32, tag="out")
                    nc.vector.tensor_copy(ot[:], ps[:])
                    nc.sync.dma_start(
                        out_d[128 * mo:128 * (mo + 1), 512 * s:512 * (s + 1)], ot[:])

    nc.compile()
    return nc


_NC_CACHE = []


def _get_nc():
    if not _NC_CACHE:
        _NC_CACHE.append(_build())
    return _NC_CACHE[0]


def kernel_run(inputs, trace=False, **kw):
    """Run on HW; returns (full_output, BassKernelResults)."""
    x = np.asarray(inputs["x"], np.float32)
    context = np.asarray(inputs["context"], np.float32)
    w_q = np.asarray(inputs["w_q"], np.float32)
    b_q = np.asarray(inputs["b_q"], np.float32)
    w_k = np.asarray(inputs["w_k"], np.float32)
    b_k = np.asarray(inputs["b_k"], np.float32)
    w_v = np.asarray(inputs["w_v"], np.float32)
    b_v = np.asarray(inputs["b_v"], np.float32)
    w_o = np.asarray(inputs["w_o"], np.float32)
    b_o = np.asarray(inputs["b_o"], np.float32)

    maps = []
    f32 = np.float32
    for c in range(8):
        b, g = c // 4, c % 4
        hs = slice(256 * g, 256 * (g + 1))
        wv_aug = np.zeros((DCTX, VAW), f32)
        bvb = np.zeros((128, VAW), f32)
        for h in range(GH):
            cs = slice(256 * g + HD * h, 256 * g + HD * (h + 1))
            wv_aug[:, VW * h:VW * h + HD] = w_v[:, cs]
            bvb[:, VW * h:VW * h + HD] = b_v[None, cs]
            bvb[:, VW * h + HD:VW * (h + 1)] = 1.0
        maps.append({
            "xT": np.ascontiguousarray(x[b].T),
            "ctxT": np.ascontiguousarray(context[b].T),
            "wq": np.ascontiguousarray(w_q[:, hs]),
            "wk": np.ascontiguousarray(w_k[:, hs]),
            "wv": wv_aug,
            "wo": np.ascontiguousarray(w_o[hs, :]),
            "bq": np.ascontiguousarray(b_q[hs].reshape(2, 128).T),
            "bk": np.ascontiguousarray(b_k[hs].reshape(2, 128).T),
            "bvb": bvb,
        })

    nc = _get_nc()
    res = bass_utils.run_bass_kernel_spmd(nc, maps, core_ids=list(range(8)),
                                          trace=trace, **kw)
    out = np.empty((B, LQ, D), np.float32)
    for b in range(B):
        acc = res.results[4 * b]["outT"].astype(np.float32)
        for g in range(1, 4):
            acc = acc + res.results[4 * b + g]["outT"]
        out[b] = acc.T + b_o[None, :]
    return out, res


def kernel(**inputs) -> np.ndarray:
    out, _ = kernel_run(inputs)
    return out



# revision 6
# speedup vs baseline: 1.2020x; 1.2020x over previous
"""Cross-attention Trainium2 kernel (nn_CrossAttention, B=2, L=2048, D=1024,
Dctx=768, 16 heads x 64).

Sharding: 8 cores = 2 (batch) x 4 (head-groups of 4 heads). Each core computes
its batch's Q/K/V projections for its 4 heads, flash-style attention in the
transposed (S^T) domain, and a partial output projection; the host sums the
head-group partials and adds b_o.

All activations live transposed on-chip (xT, ctxT, qT, kT, attnT) so every
matmul contracts over the partition dim with no on-chip transposes. The host
ships x/context pre-transposed in bf16; all matmuls run in bf16 (full PE
streaming rate, half the SBUF/DMA traffic of fp32r). The softmax denominator
comes from 64 ones-rows appended per head to V (written once by a memset, not
matmul), giving a replicated d-block in PSUM that feeds a fast-reciprocal
normalize on DVE. Output projection for the first query half is emitted
between the two attention halves so its DMA overlaps the second half.
"""
import numpy as np
from ml_dtypes import bfloat16

import concourse.bass as bass
import concourse.tile as tile
from concourse import bacc, mybir, bass_utils

BF16 = mybir.dt.bfloat16
F32 = mybir.dt.float32
EXP = mybir.ActivationFunctionType.Exp

# Problem shape (hardcoded per harness contract)
B, LQ, D = 2, 2048, 1024
DCTX = 768
NH, HD = 16, 64
SCALE = 1.0 / 8.0  # 1/sqrt(64)

# Per-core shard: 4 heads (one group), one batch
GH = 4                # heads per core
ONES = 64             # d-replication rows per head (memset, not matmul)
VW = HD + ONES        # 128: per-head width in augmented V
VAW = GH * VW         # 512
KT_Q = D // 128       # 8
KT_C = DCTX // 128    # 6
NLK = LQ // 128       # 16 key tiles
NS = LQ // 512        # 4 query 512-slices
NSB = LQ // 1024      # 2 query 1024-slices (DMA granularity)
HALF = 1024


def _build():
    nc = bacc.Bacc("TRN2", target_bir_lowering=False, debug=False,
                   enable_asserts=False, num_devices=8)

    xT_d = nc.dram_tensor("xT", (D, LQ), BF16, kind="ExternalInput").ap()
    cT_d = nc.dram_tensor("ctxT", (DCTX, LQ), BF16, kind="ExternalInput").ap()
    wq_d = nc.dram_tensor("wq", (D, 256), BF16, kind="ExternalInput").ap()
    wk_d = nc.dram_tensor("wk", (DCTX, 256), BF16, kind="ExternalInput").ap()
    wv_d = nc.dram_tensor("wv", (DCTX, 256), BF16, kind="ExternalInput").ap()
    wo_d = nc.dram_tensor("wo", (256, D), BF16, kind="ExternalInput").ap()
    bq_d = nc.dram_tensor("bq", (128, 2), F32, kind="ExternalInput").ap()
    bk_d = nc.dram_tensor("bk", (128, 2), F32, kind="ExternalInput").ap()
    bvb_d = nc.dram_tensor("bvb", (128, 256), F32, kind="ExternalInput").ap()
    out_d = nc.dram_tensor("outT", (D, LQ), F32, kind="ExternalOutput").ap()
    import os
    dbg = os.environ.get("KDBG") == "1"
    if dbg:
        dq_d = nc.dram_tensor("dbg_q", (128, 2 * LQ), BF16, kind="ExternalOutput").ap()
        dk_d = nc.dram_tensor("dbg_k", (128, 2 * LQ), BF16, kind="ExternalOutput").ap()
        dv_d = nc.dram_tensor("dbg_v", (128, NLK * VAW), BF16, kind="ExternalOutput").ap()
        da_d = nc.dram_tensor("dbg_a", (128, 2 * LQ), BF16, kind="ExternalOutput").ap()
        drd_d = nc.dram_tensor("dbg_rd", (ONES, HALF), F32, kind="ExternalOutput").ap()

    with tile.TileContext(nc) as tc:
        with tc.tile_pool(name="w", bufs=1) as wp, \
             tc.tile_pool(name="xt", bufs=10) as xtp, \
             tc.tile_pool(name="ct", bufs=12) as ctp, \
             tc.tile_pool(name="act", bufs=1) as actp, \
             tc.tile_pool(name="expp", bufs=3) as expp, \
             tc.tile_pool(name="rdp", bufs=2) as rdp, \
             tc.tile_pool(name="outp", bufs=3) as outp, \
             tc.tile_pool(name="ps_mm", bufs=2, space="PSUM") as ps_mm, \
             tc.tile_pool(name="ps_s", bufs=2, space="PSUM") as ps_s, \
             tc.tile_pool(name="ps_at", bufs=1, space="PSUM") as ps_at:

            # ---- weights / biases ----
            wk_t = wp.tile([128, KT_C * 256], BF16, tag="wk")
            nc.sync.dma_start(wk_t[:].rearrange("p (kt m) -> p kt m", m=256),
                              wk_d.rearrange("(kt p) m -> p kt m", p=128))
            bk_t = wp.tile([128, 2], F32, tag="bk")
            nc.sync.dma_start(bk_t[:], bk_d[:])
            wq_t = wp.tile([128, KT_Q * 256], BF16, tag="wq")
            nc.sync.dma_start(wq_t[:].rearrange("p (kt m) -> p kt m", m=256),
                              wq_d.rearrange("(kt p) m -> p kt m", p=128))
            bq_t = wp.tile([128, 2], F32, tag="bq")
            nc.sync.dma_start(bq_t[:], bq_d[:])
            wv_t = wp.tile([128, KT_C * 256], BF16, tag="wv")
            nc.sync.dma_start(wv_t[:].rearrange("p (kt m) -> p kt m", m=256),
                              wv_d.rearrange("(kt p) m -> p kt m", p=128))
            bvb_t = wp.tile([128, 256], F32, tag="bvb")
            nc.sync.dma_start(bvb_t[:], bvb_d[:])
            wo_t = wp.tile([128, 2 * D], BF16, tag="wo")
            nc.sync.dma_start(wo_t[:].rearrange("p (p2 m) -> p p2 m", m=1024),
                              wo_d.rearrange("(p2 p) m -> p p2 m", p=128))

            # ---- persistent activation tiles ----
            qT = [actp.tile([128, LQ], BF16, tag=f"qT{p}", name=f"qT{p}")
                  for p in range(2)]
            kT = [actp.tile([128, LQ], BF16, tag=f"kT{p}", name=f"kT{p}")
                  for p in range(2)]
            v_t = actp.tile([128, NLK * VAW], BF16, tag="v")
            aT = [actp.tile([128, LQ], BF16, tag=f"aT{p}", name=f"aT{p}")
                  for p in range(2)]

            # d-block ones: rows HD..VW of every per-head slot, written once
            ones_view = v_t[:].rearrange("p (j h w) -> p j h w",
                                         h=GH, w=VW)[:, :, :, HD:VW]
            nc.vector.memset(ones_view, 1.0)

            # ---- context DMA + K projection ----
            ct_tiles = {}
            for sb in range(NSB):
                for kt in range(KT_C):
                    t = ctp.tile([128, 1024], BF16, tag="ct")
                    nc.sync.dma_start(
                        t[:], cT_d[128 * kt:128 * (kt + 1),
                                   1024 * sb:1024 * (sb + 1)])
                    ct_tiles[(kt, sb)] = t
                for p in range(2):
                    for n in range(2):
                        ps = ps_mm.tile([128, 512], F32, tag="mm")
                        for kt in range(KT_C):
                            nc.tensor.matmul(
                                ps[:],
                                wk_t[:, 256 * kt + 128 * p:256 * kt + 128 * (p + 1)],
                                ct_tiles[(kt, sb)][:, 512 * n:512 * (n + 1)],
                                start=(kt == 0), stop=(kt == KT_C - 1))
                        nc.vector.tensor_scalar_add(
                            kT[p][:, 1024 * sb + 512 * n:1024 * sb + 512 * (n + 1)],
                            ps[:], bk_t[:, p:p + 1])

            # ---- x DMA + Q projection ----
            for sb in range(NSB):
                xt_tiles = []
                for kt in range(KT_Q):
                    t = xtp.tile([128, 1024], BF16, tag="xt")
                    nc.sync.dma_start(
                        t[:], xT_d[128 * kt:128 * (kt + 1),
                                   1024 * sb:1024 * (sb + 1)])
                    xt_tiles.append(t)
                for p in range(2):
                    for n in range(2):
                        ps = ps_mm.tile([128, 512], F32, tag="mm")
                        for kt in range(KT_Q):
                            nc.tensor.matmul(
                                ps[:],
                                wq_t[:, 256 * kt + 128 * p:256 * kt + 128 * (p + 1)],
                                xt_tiles[kt][:, 512 * n:512 * (n + 1)],
                                start=(kt == 0), stop=(kt == KT_Q - 1))
                        nc.vector.tensor_scalar_add(
                            qT[p][:, 1024 * sb + 512 * n:1024 * sb + 512 * (n + 1)],
                            ps[:], bq_t[:, p:p + 1])

            def v_chunk(j):
                # V rows for key chunk j: [128 ctx positions, 4 heads x 64]
                sb, jj = j // 8, j % 8
                ps = ps_mm.tile([128, 512], F32, tag="mm")
                for kt in range(KT_C):
                    nc.tensor.matmul(
                        ps[:, 0:256],
                        ct_tiles[(kt, sb)][:, 128 * jj:128 * (jj + 1)],
                        wv_t[:, 256 * kt:256 * (kt + 1)],
                        start=(kt == 0), stop=(kt == KT_C - 1))
                dst = v_t[:, VAW * j:VAW * (j + 1)].rearrange(
                    "p (h w) -> p h w", w=VW)[:, :, 0:HD]
                nc.vector.tensor_add(
                    dst, ps[:, 0:256].rearrange("p (h w) -> p h w", w=HD),
                    bvb_t[:].rearrange("p (h w) -> p h w", w=HD))

            def attn_block(h, half, inline_v):
                p, m = h // 2, h % 2
                r0 = 64 * m
                pa = ps_at.tile([128, HALF], F32, tag="attn")
                for j in range(NLK):
                    if inline_v:
                        v_chunk(j)
                    st = ps_s.tile([128, HALF], F32, tag="s")
                    for n in range(2):
                        nc.tensor.matmul(
                            st[:, 512 * n:512 * (n + 1)],
                            kT[p][r0:r0 + 64, 128 * j:128 * (j + 1)],
                            qT[p][r0:r0 + 64,
                                  HALF * half + 512 * n:HALF * half + 512 * (n + 1)],
                            start=True, stop=True)
                    ex = expp.tile([128, HALF], BF16, tag="expS")
                    nc.scalar.activation(ex[:], st[:], EXP, scale=SCALE)
                    for n in range(2):
                        nc.tensor.matmul(
                            pa[:, 512 * n:512 * (n + 1)],
                            v_t[:, VAW * j + VW * h:VAW * j + VW * (h + 1)],
                            ex[:, 512 * n:512 * (n + 1)],
                            start=(j == 0), stop=(j == NLK - 1))
                # normalize: attnT = attnU * (1/d); d replicated in pa[64:128]
                dsb = rdp.tile([ONES, HALF], F32, tag="dsb")
                rd = rdp.tile([ONES, HALF], F32, tag="rd")
                nc.vector.tensor_copy(dsb[:], pa[64:128, :])
                nc.vector.reciprocal_approx_fast(rd[:], dsb[:])
                if dbg and h == 0 and half == 0:
                    nc.sync.dma_start(drd_d[:], rd[:])
                cols = slice(HALF * half, HALF * (half + 1))
                nc.vector.tensor_mul(aT[p][r0:r0 + 64, cols], pa[0:64, :], rd[:])

            def out_proj(s):
                # outT[:, 512-slice s] partial for this head group
                for mo in range(D // 128):
                    ps = ps_mm.tile([128, 512], F32, tag="mm")
                    for p in range(2):
                        nc.tensor.matmul(
                            ps[:], wo_t[:, D * p + 128 * mo:D * p + 128 * (mo + 1)],
                            aT[p][:, 512 * s:512 * (s + 1)],
                            start=(p == 0), stop=(p == 1))
                    ot = outp.tile([128, 512], F32, tag="out")
                    nc.vector.tensor_copy(ot[:], ps[:])
                    nc.sync.dma_start(
                        out_d[128 * mo:128 * (mo + 1), 512 * s:512 * (s + 1)],
                        ot[:])

            # ---- attention + output projection ----
            for half in range(2):
                for h in range(GH):
                    attn_block(h, half, inline_v=(half == 0 and h == 0))
                for s in (2 * half, 2 * half + 1):
                    out_proj(s)

            if dbg:
                for p in range(2):
                    nc.sync.dma_start(dq_d[:, LQ * p:LQ * (p + 1)], qT[p][:])
                    nc.sync.dma_start(dk_d[:, LQ * p:LQ * (p + 1)], kT[p][:])
                    nc.sync.dma_start(da_d[:, LQ * p:LQ * (p + 1)], aT[p][:])
                nc.sync.dma_start(dv_d[:], v_t[:])

    nc.compile()
    return nc


_NC_CACHE = []


def _get_nc():
    if not _NC_CACHE:
        _NC_CACHE.append(_build())
    return _NC_CACHE[0]


def kernel_run(inputs, trace=False, **kw):
    """Run on HW; returns (full_output, BassKernelResults)."""
    x = np.asarray(inputs["x"], np.float32)
    context = np.asarray(inputs["context"], np.float32)
    w_q = np.asarray(inputs["w_q"], np.float32)
    b_q = np.asarray(inputs["b_q"], np.float32)
    w_k = np.asarray(inputs["w_k"], np.float32)
    b_k = np.asarray(inputs["b_k"], np.float32)
    w_v = np.asarray(inputs["w_v"], np.float32)
    b_v = np.asarray(inputs["b_v"], np.float32)
    w_o = np.asarray(inputs["w_o"], np.float32)
    b_o = np.asarray(inputs["b_o"], np.float32)

    xT = [np.ascontiguousarray(x[b].T).astype(bfloat16) for b in range(B)]
    cT = [np.ascontiguousarray(context[b].T).astype(bfloat16) for b in range(B)]

    maps = []
    for c in range(8):
        b, g = c // 4, c % 4
        hs = slice(256 * g, 256 * (g + 1))
        maps.append({
            "xT": xT[b],
            "ctxT": cT[b],
            "wq": np.ascontiguousarray(w_q[:, hs]).astype(bfloat16),
            "wk": np.ascontiguousarray(w_k[:, hs]).astype(bfloat16),
            "wv": np.ascontiguousarray(w_v[:, hs]).astype(bfloat16),
            "wo": np.ascontiguousarray(w_o[hs, :]).astype(bfloat16),
            "bq": np.ascontiguousarray(b_q[hs].reshape(2, 128).T.astype(np.float32)),
            "bk": np.ascontiguousarray(b_k[hs].reshape(2, 128).T.astype(np.float32)),
            "bvb": np.broadcast_to(b_v[hs].astype(np.float32), (128, 256)).copy(),
        })

    nc = _get_nc()
    res = bass_utils.run_bass_kernel_spmd(nc, maps, core_ids=list(range(8)),
                                          trace=trace, **kw)
    out = np.empty((B, LQ, D), np.float32)
    for b in range(B):
        acc = res.results[4 * b]["outT"].astype(np.float32)
        for g in range(1, 4):
            acc = acc + res.results[4 * b + g]["outT"]
        out[b] = acc.T + b_o[None, :]
    return out, res


def kernel(**inputs) -> np.ndarray:
    out, _ = kernel_run(inputs)
    return out


# revision 11
# speedup vs baseline: 1.5160x; 1.2612x over previous
"""Cross-attention Trainium2 kernel (nn_CrossAttention, B=2, L=2048, D=1024,
Dctx=768, 16 heads x 64).

Sharding: 8 cores = 2 (batch) x 4 (head-groups of 4 heads). Each core computes
its batch's Q/K/V projections for its 4 heads, flash-style attention in the
transposed (S^T) domain, and a partial output projection; the host sums the
head-group partials and adds b_o.

All activations live transposed on-chip (xT, ctxT, qT, kT, attnT) so every
matmul contracts over the partition dim with no on-chip transposes. The host
ships x/context pre-transposed in fp16; all matmuls run in fp16 (full PE
streaming rate, half the SBUF/DMA traffic of fp32r, 2 more mantissa bits than
bf16). The kernel is software-pipelined around the Scalar engine's softmax
exp, which is the binding resource: attnV matmuls are emitted one key-tile
behind the scores so the PE never waits on exp, and all projection / output
matmul chains are spread as filler work inside the attention loops to keep
the tensor engine continuously busy (which also holds its fast p-state).
The softmax denominator comes from 64 ones-rows appended per head to V
(written once by a memset), giving a replicated d-block in PSUM; each
attention block's PSUM tile is copied whole to SBUF so the PSUM bank frees
early, then a fast-reciprocal normalize runs on DVE.
"""
import numpy as np

import concourse.bass as bass
import concourse.tile as tile
from concourse import bacc, mybir, bass_utils

FP16 = mybir.dt.float16
F32 = mybir.dt.float32
EXP = mybir.ActivationFunctionType.Exp

# Problem shape (hardcoded per harness contract)
B, LQ, D = 2, 2048, 1024
DCTX = 768
NH, HD = 16, 64
SCALE = 1.0 / 8.0  # 1/sqrt(64)

# Per-core shard: 4 heads (one group), one batch
GH = 4                # heads per core
ONES = 64             # d-replication rows per head (memset, not matmul)
VW = HD + ONES        # 128: per-head width in augmented V
VAW = GH * VW         # 512
KT_Q = D // 128       # 8
KT_C = DCTX // 128    # 6
NLK = LQ // 128       # 16 key tiles
NSB = LQ // 1024      # 2 query 1024-slices (DMA granularity)
HALF = 1024


def _build():
    nc = bacc.Bacc("TRN2", target_bir_lowering=False, debug=False,
                   enable_asserts=False, num_devices=8)

    xT_d = nc.dram_tensor("xT", (D, LQ), FP16, kind="ExternalInput").ap()
    cT_d = nc.dram_tensor("ctxT", (DCTX, LQ), FP16, kind="ExternalInput").ap()
    wq_d = nc.dram_tensor("wq", (D, 256), FP16, kind="ExternalInput").ap()
    wk_d = nc.dram_tensor("wk", (DCTX, 256), FP16, kind="ExternalInput").ap()
    wv_d = nc.dram_tensor("wv", (DCTX, 256), FP16, kind="ExternalInput").ap()
    wo_d = nc.dram_tensor("wo", (256, D), FP16, kind="ExternalInput").ap()
    bq_d = nc.dram_tensor("bq", (128, 2), F32, kind="ExternalInput").ap()
    bk_d = nc.dram_tensor("bk", (128, 2), F32, kind="ExternalInput").ap()
    bvb_d = nc.dram_tensor("bvb", (128, 256), F32, kind="ExternalInput").ap()
    out_d = nc.dram_tensor("outT", (D, LQ), F32, kind="ExternalOutput").ap()
    import os
    dbg = os.environ.get("KDBG") == "1"
    if dbg:
        dq_d = nc.dram_tensor("dbg_q", (128, 2 * LQ), FP16, kind="ExternalOutput").ap()
        dk_d = nc.dram_tensor("dbg_k", (128, 2 * LQ), FP16, kind="ExternalOutput").ap()
        dv_d = nc.dram_tensor("dbg_v", (128, NLK * VAW), FP16, kind="ExternalOutput").ap()
        da_d = nc.dram_tensor("dbg_a", (128, 2 * LQ), FP16, kind="ExternalOutput").ap()
        drd_d = nc.dram_tensor("dbg_rd", (ONES, HALF), F32, kind="ExternalOutput").ap()

    with tile.TileContext(nc) as tc:
        with tc.tile_pool(name="w", bufs=1) as wp, \
             tc.tile_pool(name="xt", bufs=16) as xtp, \
             tc.tile_pool(name="ct", bufs=12) as ctp, \
             tc.tile_pool(name="act", bufs=1) as actp, \
             tc.tile_pool(name="expp", bufs=4) as expp, \
             tc.tile_pool(name="rdp", bufs=2) as rdp, \
             tc.tile_pool(name="outp", bufs=3) as outp, \
             tc.tile_pool(name="ps_mm", bufs=2, space="PSUM") as ps_mm, \
             tc.tile_pool(name="ps_s", bufs=2, space="PSUM") as ps_s, \
             tc.tile_pool(name="ps_at", bufs=1, space="PSUM") as ps_at:

            # ---- weight/bias and streamed-input DMAs (in first-use order) ----
            wk_t = wp.tile([128, KT_C * 256], FP16, tag="wk")
            nc.sync.dma_start(wk_t[:].rearrange("p (kt m) -> p kt m", m=256),
                              wk_d.rearrange("(kt p) m -> p kt m", p=128))
            bk_t = wp.tile([128, 2], F32, tag="bk")
            nc.sync.dma_start(bk_t[:], bk_d[:])
            ct_tiles = {}
            for sb in range(NSB):
                for kt in range(KT_C):
                    t = ctp.tile([128, 1024], FP16, tag="ct")
                    nc.sync.dma_start(
                        t[:], cT_d[128 * kt:128 * (kt + 1),
                                   1024 * sb:1024 * (sb + 1)])
                    ct_tiles[(kt, sb)] = t
                if sb == 0:
                    wq_t = wp.tile([128, KT_Q * 256], FP16, tag="wq")
                    nc.sync.dma_start(
                        wq_t[:].rearrange("p (kt m) -> p kt m", m=256),
                        wq_d.rearrange("(kt p) m -> p kt m", p=128))
                    bq_t = wp.tile([128, 2], F32, tag="bq")
                    nc.sync.dma_start(bq_t[:], bq_d[:])
            xt_tiles = {}
            for sb in range(NSB):
                for kt in range(KT_Q):
                    t = xtp.tile([128, 1024], FP16, tag="xt")
                    nc.sync.dma_start(
                        t[:], xT_d[128 * kt:128 * (kt + 1),
                                   1024 * sb:1024 * (sb + 1)])
                    xt_tiles[(kt, sb)] = t
            wv_t = wp.tile([128, KT_C * 256], FP16, tag="wv")
            nc.sync.dma_start(wv_t[:].rearrange("p (kt m) -> p kt m", m=256),
                              wv_d.rearrange("(kt p) m -> p kt m", p=128))
            bvb_t = wp.tile([128, 256], F32, tag="bvb")
            nc.sync.dma_start(bvb_t[:], bvb_d[:])
            wo_t = wp.tile([128, 2 * D], FP16, tag="wo")
            nc.sync.dma_start(wo_t[:].rearrange("p (p2 m) -> p p2 m", m=1024),
                              wo_d.rearrange("(p2 p) m -> p p2 m", p=128))

            # ---- persistent activation tiles ----
            qT = [actp.tile([128, LQ], FP16, tag=f"qT{p}", name=f"qT{p}")
                  for p in range(2)]
            kT = [actp.tile([128, LQ], FP16, tag=f"kT{p}", name=f"kT{p}")
                  for p in range(2)]
            v_t = actp.tile([128, NLK * VAW], FP16, tag="v")
            aT = [actp.tile([128, LQ], FP16, tag=f"aT{p}", name=f"aT{p}")
                  for p in range(2)]

            # d-block ones: rows HD..VW of every per-head slot, written once
            ones_view = v_t[:].rearrange("p (j h w) -> p j h w",
                                         h=GH, w=VW)[:, :, :, HD:VW]
            nc.vector.memset(ones_view, 1.0)

            # ---- projection chain emitters (each is one PSUM accumulation) ----
            def k_chain(p, sb, n):
                ps = ps_mm.tile([128, 512], F32, tag="mm")
                for kt in range(KT_C):
                    nc.tensor.matmul(
                        ps[:], wk_t[:, 256 * kt + 128 * p:256 * kt + 128 * (p + 1)],
                        ct_tiles[(kt, sb)][:, 512 * n:512 * (n + 1)],
                        start=(kt == 0), stop=(kt == KT_C - 1))
                nc.vector.tensor_scalar_add(
                    kT[p][:, 1024 * sb + 512 * n:1024 * sb + 512 * (n + 1)],
                    ps[:], bk_t[:, p:p + 1])

            def q_chain(p, sb, n):
                ps = ps_mm.tile([128, 512], F32, tag="mm")
                for kt in range(KT_Q):
                    nc.tensor.matmul(
                        ps[:], wq_t[:, 256 * kt + 128 * p:256 * kt + 128 * (p + 1)],
                        xt_tiles[(kt, sb)][:, 512 * n:512 * (n + 1)],
                        start=(kt == 0), stop=(kt == KT_Q - 1))
                nc.vector.tensor_scalar_add(
                    qT[p][:, 1024 * sb + 512 * n:1024 * sb + 512 * (n + 1)],
                    ps[:], bq_t[:, p:p + 1])

            def v_chunk(j):
                # V rows for key chunk j: [128 ctx positions, 4 heads x 64]
                sb, jj = j // 8, j % 8
                ps = ps_mm.tile([128, 512], F32, tag="mm")
                for kt in range(KT_C):
                    nc.tensor.matmul(
                        ps[:, 0:256],
                        ct_tiles[(kt, sb)][:, 128 * jj:128 * (jj + 1)],
                        wv_t[:, 256 * kt:256 * (kt + 1)],
                        start=(kt == 0), stop=(kt == KT_C - 1))
                dst = v_t[:, VAW * j:VAW * (j + 1)].rearrange(
                    "p (h w) -> p h w", w=VW)[:, :, 0:HD]
                nc.vector.tensor_add(
                    dst, ps[:, 0:256].rearrange("p (h w) -> p h w", w=HD),
                    bvb_t[:].rearrange("p (h w) -> p h w", w=HD))

            def out_unit(s, mo):
                # outT[128-row slice mo, 512-col slice s] partial
                ps = ps_mm.tile([128, 512], F32, tag="mm")
                for p in range(2):
                    nc.tensor.matmul(
                        ps[:], wo_t[:, D * p + 128 * mo:D * p + 128 * (mo + 1)],
                        aT[p][:, 512 * s:512 * (s + 1)],
                        start=(p == 0), stop=(p == 1))
                ot = outp.tile([128, 512], F32, tag="out")
                nc.vector.tensor_copy(ot[:], ps[:])
                nc.sync.dma_start(
                    out_d[128 * mo:128 * (mo + 1), 512 * s:512 * (s + 1)],
                    ot[:])

            # ---- prologue: everything attention block (0,0) needs ----
            for n in range(2):
                k_chain(0, 0, n)
            for n in range(2):
                q_chain(0, 0, n)
            for n in range(2):
                k_chain(0, 1, n)

            # ---- filler schedules per attention block ----
            # Every chain must be emitted strictly before its first consumer:
            # kT[1]/qT[1] sb0 before block (0,2); kT[1] sb1 before (0,2) j=8;
            # qT[0] sb1 before (1,0); qT[1] sb1 before (1,2); out s0/s1 after
            # all half-0 normalizes (i.e. inside half-1 blocks).
            fillers = {
                (0, 0): [],  # V chunks are inlined per-j here
                (0, 1): [lambda n=n: k_chain(1, 0, n) for n in range(2)]
                        + [lambda n=n: q_chain(1, 0, n) for n in range(2)],
                (0, 2): [lambda n=n: k_chain(1, 1, n) for n in range(2)]
                        + [lambda: q_chain(0, 1, 0)],
                (0, 3): [lambda: q_chain(0, 1, 1), lambda: q_chain(1, 1, 0)],
                (1, 0): [lambda: q_chain(1, 1, 1)]
                        + [lambda mo=mo: out_unit(0, mo) for mo in range(3)],
                (1, 1): [lambda mo=mo: out_unit(0, mo) for mo in range(3, 8)],
                (1, 2): [lambda mo=mo: out_unit(1, mo) for mo in range(5)],
                (1, 3): [lambda mo=mo: out_unit(1, mo) for mo in range(5, 8)],
            }

            def attn_block(h, half):
                p, m = h // 2, h % 2
                r0 = 64 * m
                inline_v = (half == 0 and h == 0)
                fl = list(fillers[(half, h)])
                # spread fillers evenly across the 16 j-iterations
                slots = {}
                for i in range(len(fl)):
                    slots[1 + (i * 14) // max(len(fl), 1)] = fl[i]
                pa = ps_at.tile([128, HALF], F32, tag="attn")
                ex_tiles = {}
                for j in range(NLK):
                    if inline_v:
                        v_chunk(j)
                    st = ps_s.tile([128, HALF], F32, tag="s")
                    for n in range(2):
                        nc.tensor.matmul(
                            st[:, 512 * n:512 * (n + 1)],
                            kT[p][r0:r0 + 64, 128 * j:128 * (j + 1)],
                            qT[p][r0:r0 + 64,
                                  HALF * half + 512 * n:HALF * half + 512 * (n + 1)],
                            start=True, stop=True)
                    ex = expp.tile([128, HALF], FP16, tag="expS")
                    nc.scalar.activation(ex[:], st[:], EXP, scale=SCALE)
                    ex_tiles[j] = ex
                    if j in slots:
                        slots[j]()
                    if j > 0:
                        attn_v(pa, h, j - 1, ex_tiles.pop(j - 1))
                attn_v(pa, h, NLK - 1, ex_tiles.pop(NLK - 1))
                # normalize: copy PSUM out whole (frees the bank), then
                # attnT = attnU * (1/d) with d replicated in rows 64:128
                dsb = rdp.tile([ONES, HALF], F32, tag="dsb")
                nc.vector.tensor_copy(dsb[:], pa[64:128, :])
                rd = rdp.tile([ONES, HALF], F32, tag="rd")
                nc.vector.reciprocal_approx_fast(rd[:], dsb[:])
                if dbg and h == 0 and half == 0:
                    nc.sync.dma_start(drd_d[:], rd[:])
                cols = slice(HALF * half, HALF * (half + 1))
                nc.vector.tensor_mul(aT[p][r0:r0 + 64, cols], pa[0:64, :], rd[:])

            def attn_v(pa, h, j, ex):
                for n in range(2):
                    nc.tensor.matmul(
                        pa[:, 512 * n:512 * (n + 1)],
                        v_t[:, VAW * j + VW * h:VAW * j + VW * (h + 1)],
                        ex[:, 512 * n:512 * (n + 1)],
                        start=(j == 0), stop=(j == NLK - 1))

            # ---- attention + interleaved projections / output ----
            for half in range(2):
                for h in range(GH):
                    attn_block(h, half)
            for s in (2, 3):
                for mo in range(D // 128):
                    out_unit(s, mo)

            if dbg:
                for p in range(2):
                    nc.sync.dma_start(dq_d[:, LQ * p:LQ * (p + 1)], qT[p][:])
                    nc.sync.dma_start(dk_d[:, LQ * p:LQ * (p + 1)], kT[p][:])
                    nc.sync.dma_start(da_d[:, LQ * p:LQ * (p + 1)], aT[p][:])
                nc.sync.dma_start(dv_d[:], v_t[:])

    nc.compile()
    return nc


_NC_CACHE = []


def _get_nc():
    if not _NC_CACHE:
        _NC_CACHE.append(_build())
    return _NC_CACHE[0]


def kernel_run(inputs, trace=False, **kw):
    """Run on HW; returns (full_output, BassKernelResults)."""
    x = np.asarray(inputs["x"], np.float32)
    context = np.asarray(inputs["context"], np.float32)
    w_q = np.asarray(inputs["w_q"], np.float32)
    b_q = np.asarray(inputs["b_q"], np.float32)
    w_k = np.asarray(inputs["w_k"], np.float32)
    b_k = np.asarray(inputs["b_k"], np.float32)
    w_v = np.asarray(inputs["w_v"], np.float32)
    b_v = np.asarray(inputs["b_v"], np.float32)
    w_o = np.asarray(inputs["w_o"], np.float32)
    b_o = np.asarray(inputs["b_o"], np.float32)

    f16 = np.float16
    xT = [np.ascontiguousarray(x[b].T).astype(f16) for b in range(B)]
    cT = [np.ascontiguousarray(context[b].T).astype(f16) for b in range(B)]

    maps = []
    for c in range(8):
        b, g = c // 4, c % 4
        hs = slice(256 * g, 256 * (g + 1))
        maps.append({
            "xT": xT[b],
            "ctxT": cT[b],
            "wq": np.ascontiguousarray(w_q[:, hs]).astype(f16),
            "wk": np.ascontiguousarray(w_k[:, hs]).astype(f16),
            "wv": np.ascontiguousarray(w_v[:, hs]).astype(f16),
            "wo": np.ascontiguousarray(w_o[hs, :]).astype(f16),
            "bq": np.ascontiguousarray(b_q[hs].reshape(2, 128).T.astype(np.float32)),
            "bk": np.ascontiguousarray(b_k[hs].reshape(2, 128).T.astype(np.float32)),
            "bvb": np.broadcast_to(b_v[hs].astype(np.float32), (128, 256)).copy(),
        })

    nc = _get_nc()
    res = bass_utils.run_bass_kernel_spmd(nc, maps, core_ids=list(range(8)),
                                          trace=trace, **kw)
    out = np.empty((B, LQ, D), np.float32)
    for b in range(B):
        acc = res.results[4 * b]["outT"].astype(np.float32)
        for g in range(1, 4):
            acc = acc + res.results[4 * b + g]["outT"]
        out[b] = acc.T + b_o[None, :]
    return out, res


def kernel(**inputs) -> np.ndarray:
    out, _ = kernel_run(inputs)
    return out


# revision 12
# speedup vs baseline: 1.6583x; 1.0938x over previous
"""Cross-attention Trainium2 kernel (nn_CrossAttention, B=2, L=2048, D=1024,
Dctx=768, 16 heads x 64).

Sharding: 8 cores = 2 (batch) x 4 (head-groups of 4 heads). Each core computes
its batch's Q/K/V projections for its 4 heads, flash-style attention in the
transposed (S^T) domain, and a partial output projection; the host sums the
head-group partials and adds b_o.

All activations live transposed on-chip (xT, ctxT, qT, kT, attnT) so every
matmul contracts over the partition dim with no on-chip transposes. The host
ships x/context pre-transposed in fp16; all matmuls run in fp16 (full PE
streaming rate, half the SBUF/DMA traffic of fp32r, 2 more mantissa bits than
bf16). The kernel is software-pipelined around the Scalar engine's softmax
exp, which is the binding resource: attnV matmuls are emitted two key-tiles
behind the scores so the PE never waits on exp, and all projection / output
matmul chains are spread as filler work at explicit slots inside the
attention loops to keep the tensor engine continuously busy (which also
holds its fast p-state). The softmax denominator comes from 64 ones-rows
appended per head to V (written once by a memset); the d-block is copied to
SBUF, inverted with the fast-reciprocal DVE op, and applied to the PSUM
attention tile. The final output projection writes [128,1024] tiles with the
two PSUM->SBUF copies split across the Scalar and Vector engines.
"""
import numpy as np

import concourse.bass as bass
import concourse.tile as tile
from concourse import bacc, mybir, bass_utils

FP16 = mybir.dt.float16
F32 = mybir.dt.float32
EXP = mybir.ActivationFunctionType.Exp
CPY = mybir.ActivationFunctionType.Copy

# Problem shape (hardcoded per harness contract)
B, LQ, D = 2, 2048, 1024
DCTX = 768
NH, HD = 16, 64
SCALE = 1.0 / 8.0  # 1/sqrt(64)

# Per-core shard: 4 heads (one group), one batch
GH = 4                # heads per core
ONES = 64             # d-replication rows per head (memset, not matmul)
VW = HD + ONES        # 128: per-head width in augmented V
VAW = GH * VW         # 512
KT_Q = D // 128       # 8
KT_C = DCTX // 128    # 6
NLK = LQ // 128       # 16 key tiles
NSB = LQ // 1024      # 2 query 1024-slices (DMA granularity)
HALF = 1024
LAG = 2               # attnV trails scores by this many key tiles


def _build():
    nc = bacc.Bacc("TRN2", target_bir_lowering=False, debug=False,
                   enable_asserts=False, num_devices=8)

    xT_d = nc.dram_tensor("xT", (D, LQ), FP16, kind="ExternalInput").ap()
    cT_d = nc.dram_tensor("ctxT", (DCTX, LQ), FP16, kind="ExternalInput").ap()
    wq_d = nc.dram_tensor("wq", (D, 256), FP16, kind="ExternalInput").ap()
    wk_d = nc.dram_tensor("wk", (DCTX, 256), FP16, kind="ExternalInput").ap()
    wv_d = nc.dram_tensor("wv", (DCTX, 256), FP16, kind="ExternalInput").ap()
    wo_d = nc.dram_tensor("wo", (256, D), FP16, kind="ExternalInput").ap()
    bq_d = nc.dram_tensor("bq", (128, 2), F32, kind="ExternalInput").ap()
    bk_d = nc.dram_tensor("bk", (128, 2), F32, kind="ExternalInput").ap()
    bvb_d = nc.dram_tensor("bvb", (128, 256), F32, kind="ExternalInput").ap()
    out_d = nc.dram_tensor("outT", (D, LQ), F32, kind="ExternalOutput").ap()
    import os
    dbg = os.environ.get("KDBG") == "1"
    if dbg:
        dq_d = nc.dram_tensor("dbg_q", (128, 2 * LQ), FP16, kind="ExternalOutput").ap()
        dk_d = nc.dram_tensor("dbg_k", (128, 2 * LQ), FP16, kind="ExternalOutput").ap()
        dv_d = nc.dram_tensor("dbg_v", (128, NLK * VAW), FP16, kind="ExternalOutput").ap()
        da_d = nc.dram_tensor("dbg_a", (128, 2 * LQ), FP16, kind="ExternalOutput").ap()
        drd_d = nc.dram_tensor("dbg_rd", (ONES, HALF), F32, kind="ExternalOutput").ap()

    with tile.TileContext(nc) as tc:
        with tc.tile_pool(name="w", bufs=1) as wp, \
             tc.tile_pool(name="xt", bufs=16) as xtp, \
             tc.tile_pool(name="ct", bufs=12) as ctp, \
             tc.tile_pool(name="act", bufs=1) as actp, \
             tc.tile_pool(name="expp", bufs=5) as expp, \
             tc.tile_pool(name="rdp", bufs=2) as rdp, \
             tc.tile_pool(name="outp", bufs=3) as outp, \
             tc.tile_pool(name="ps_mm", bufs=2, space="PSUM") as ps_mm, \
             tc.tile_pool(name="ps_s", bufs=2, space="PSUM") as ps_s, \
             tc.tile_pool(name="ps_at", bufs=1, space="PSUM") as ps_at:

            # ---- DMAs in first-use order ----
            wk_t = wp.tile([128, KT_C * 256], FP16, tag="wk")
            nc.sync.dma_start(wk_t[:].rearrange("p (kt m) -> p kt m", m=256),
                              wk_d.rearrange("(kt p) m -> p kt m", p=128))
            bk_t = wp.tile([128, 2], F32, tag="bk")
            nc.sync.dma_start(bk_t[:], bk_d[:])
            ct_tiles = {}
            for kt in range(KT_C):
                t = ctp.tile([128, 1024], FP16, tag="ct")
                nc.sync.dma_start(t[:], cT_d[128 * kt:128 * (kt + 1), 0:1024])
                ct_tiles[(kt, 0)] = t
            wv_t = wp.tile([128, KT_C * 256], FP16, tag="wv")
            nc.sync.dma_start(wv_t[:].rearrange("p (kt m) -> p kt m", m=256),
                              wv_d.rearrange("(kt p) m -> p kt m", p=128))
            bvb_t = wp.tile([128, 256], F32, tag="bvb")
            nc.sync.dma_start(bvb_t[:], bvb_d[:])
            wq_t = wp.tile([128, KT_Q * 256], FP16, tag="wq")
            nc.sync.dma_start(wq_t[:].rearrange("p (kt m) -> p kt m", m=256),
                              wq_d.rearrange("(kt p) m -> p kt m", p=128))
            bq_t = wp.tile([128, 2], F32, tag="bq")
            nc.sync.dma_start(bq_t[:], bq_d[:])
            xt_tiles = {}
            for kt in range(KT_Q):
                t = xtp.tile([128, 1024], FP16, tag="xt")
                nc.sync.dma_start(t[:], xT_d[128 * kt:128 * (kt + 1), 0:1024])
                xt_tiles[(kt, 0)] = t
            for kt in range(KT_C):
                t = ctp.tile([128, 1024], FP16, tag="ct")
                nc.sync.dma_start(t[:], cT_d[128 * kt:128 * (kt + 1), 1024:2048])
                ct_tiles[(kt, 1)] = t
            for kt in range(KT_Q):
                t = xtp.tile([128, 1024], FP16, tag="xt")
                nc.sync.dma_start(t[:], xT_d[128 * kt:128 * (kt + 1), 1024:2048])
                xt_tiles[(kt, 1)] = t
            wo_t = wp.tile([128, 2 * D], FP16, tag="wo")
            nc.sync.dma_start(wo_t[:].rearrange("p (p2 m) -> p p2 m", m=1024),
                              wo_d.rearrange("(p2 p) m -> p p2 m", p=128))

            # ---- persistent activation tiles ----
            qT = [actp.tile([128, LQ], FP16, tag=f"qT{p}", name=f"qT{p}")
                  for p in range(2)]
            kT = [actp.tile([128, LQ], FP16, tag=f"kT{p}", name=f"kT{p}")
                  for p in range(2)]
            v_t = actp.tile([128, NLK * VAW], FP16, tag="v")
            aT = [actp.tile([128, LQ], FP16, tag=f"aT{p}", name=f"aT{p}")
                  for p in range(2)]

            # d-block ones: rows HD..VW of every per-head slot, written once
            ones_view = v_t[:].rearrange("p (j h w) -> p j h w",
                                         h=GH, w=VW)[:, :, :, HD:VW]
            nc.vector.memset(ones_view, 1.0)

            # ---- chain emitters (each is one PSUM accumulation) ----
            def k_chain(p, sb, n):
                ps = ps_mm.tile([128, 512], F32, tag="mm")
                for kt in range(KT_C):
                    nc.tensor.matmul(
                        ps[:], wk_t[:, 256 * kt + 128 * p:256 * kt + 128 * (p + 1)],
                        ct_tiles[(kt, sb)][:, 512 * n:512 * (n + 1)],
                        start=(kt == 0), stop=(kt == KT_C - 1))
                nc.vector.tensor_scalar_add(
                    kT[p][:, 1024 * sb + 512 * n:1024 * sb + 512 * (n + 1)],
                    ps[:], bk_t[:, p:p + 1])

            def q_chain(p, sb, n):
                ps = ps_mm.tile([128, 512], F32, tag="mm")
                for kt in range(KT_Q):
                    nc.tensor.matmul(
                        ps[:], wq_t[:, 256 * kt + 128 * p:256 * kt + 128 * (p + 1)],
                        xt_tiles[(kt, sb)][:, 512 * n:512 * (n + 1)],
                        start=(kt == 0), stop=(kt == KT_Q - 1))
                nc.vector.tensor_scalar_add(
                    qT[p][:, 1024 * sb + 512 * n:1024 * sb + 512 * (n + 1)],
                    ps[:], bq_t[:, p:p + 1])

            def v_chunk(j):
                # V rows for key chunk j: [128 ctx positions, 4 heads x 64]
                sb, jj = j // 8, j % 8
                ps = ps_mm.tile([128, 512], F32, tag="mm")
                for kt in range(KT_C):
                    nc.tensor.matmul(
                        ps[:, 0:256],
                        ct_tiles[(kt, sb)][:, 128 * jj:128 * (jj + 1)],
                        wv_t[:, 256 * kt:256 * (kt + 1)],
                        start=(kt == 0), stop=(kt == KT_C - 1))
                dst = v_t[:, VAW * j:VAW * (j + 1)].rearrange(
                    "p (h w) -> p h w", w=VW)[:, :, 0:HD]
                nc.vector.tensor_add(
                    dst, ps[:, 0:256].rearrange("p (h w) -> p h w", w=HD),
                    bvb_t[:].rearrange("p (h w) -> p h w", w=HD))

            def out_mm(s, mo):
                ps = ps_mm.tile([128, 512], F32, tag="mm")
                for p in range(2):
                    nc.tensor.matmul(
                        ps[:], wo_t[:, D * p + 128 * mo:D * p + 128 * (mo + 1)],
                        aT[p][:, 512 * s:512 * (s + 1)],
                        start=(p == 0), stop=(p == 1))
                return ps

            def out_unit(s, mo):
                # one [128,512] output slice: matmul + DVE copy + DMA
                ps = out_mm(s, mo)
                ot = outp.tile([128, 512], F32, tag="out")
                nc.vector.tensor_copy(ot[:], ps[:])
                nc.sync.dma_start(
                    out_d[128 * mo:128 * (mo + 1), 512 * s:512 * (s + 1)],
                    ot[:])

            def out_tail(mo):
                # [128,1024] tile for query cols 1024:2048; the two PSUM
                # copies run on Scalar and DVE in parallel, single DMA
                ot = outp.tile([128, 1024], F32, tag="outw")
                ps2 = out_mm(2, mo)
                nc.scalar.activation(ot[:, 0:512], ps2[:], CPY)
                ps3 = out_mm(3, mo)
                nc.vector.tensor_copy(ot[:, 512:1024], ps3[:])
                nc.sync.dma_start(
                    out_d[128 * mo:128 * (mo + 1), 1024:2048], ot[:])

            # ---- per-block filler slots: {j: closure} ----
            # Every chain must land strictly before its first consumer:
            # kT[0] sb1 before (0,0) j=8; kT[1]/qT[1] sb0 before (0,2);
            # kT[1] sb1 before (0,2) j=8; qT[0] sb1 before (1,0); qT[1] sb1
            # before (1,2); out s0/s1 after all half-0 normalizes.
            fillers = {
                (0, 0): {1: lambda: k_chain(0, 1, 0),
                         3: lambda: k_chain(0, 1, 1)},
                (0, 1): {1: lambda: k_chain(1, 0, 0),
                         3: lambda: k_chain(1, 0, 1),
                         6: lambda: q_chain(1, 0, 0),
                         9: lambda: q_chain(1, 0, 1),
                         12: lambda: q_chain(0, 1, 0)},
                (0, 2): {1: lambda: k_chain(1, 1, 0),
                         4: lambda: k_chain(1, 1, 1),
                         8: lambda: q_chain(0, 1, 1),
                         12: lambda: q_chain(1, 1, 0)},
                (0, 3): {4: lambda: q_chain(1, 1, 1)},
                (1, 0): {1 + 4 * i: (lambda mo=i: out_unit(0, mo))
                         for i in range(4)},
                (1, 1): {1 + 4 * i: (lambda mo=mo: out_unit(0, mo))
                         for i, mo in enumerate(range(4, 8))},
                (1, 2): {1 + 4 * i: (lambda mo=i: out_unit(1, mo))
                         for i in range(4)},
                (1, 3): {1 + 4 * i: (lambda mo=mo: out_unit(1, mo))
                         for i, mo in enumerate(range(4, 8))},
            }

            def attn_v(pa, h, j, ex):
                for n in range(2):
                    nc.tensor.matmul(
                        pa[:, 512 * n:512 * (n + 1)],
                        v_t[:, VAW * j + VW * h:VAW * j + VW * (h + 1)],
                        ex[:, 512 * n:512 * (n + 1)],
                        start=(j == 0), stop=(j == NLK - 1))

            def attn_block(h, half):
                p, m = h // 2, h % 2
                r0 = 64 * m
                inline_v = (half == 0 and h == 0)
                slots = fillers[(half, h)]
                pa = ps_at.tile([128, HALF], F32, tag="attn")
                ex_tiles = {}
                for j in range(NLK):
                    if inline_v and j >= 6:
                        v_chunk(j)
                    st = ps_s.tile([128, HALF], F32, tag="s")
                    for n in range(2):
                        nc.tensor.matmul(
                            st[:, 512 * n:512 * (n + 1)],
                            kT[p][r0:r0 + 64, 128 * j:128 * (j + 1)],
                            qT[p][r0:r0 + 64,
                                  HALF * half + 512 * n:HALF * half + 512 * (n + 1)],
                            start=True, stop=True)
                    ex = expp.tile([128, HALF], FP16, tag="expS")
                    nc.scalar.activation(ex[:], st[:], EXP, scale=SCALE)
                    ex_tiles[j] = ex
                    if j in slots:
                        slots[j]()
                    if j >= LAG:
                        attn_v(pa, h, j - LAG, ex_tiles.pop(j - LAG))
                for j in range(NLK - LAG, NLK):
                    attn_v(pa, h, j, ex_tiles.pop(j))
                # normalize: attnT = attnU * (1/d); d replicated in pa[64:128]
                dsb = rdp.tile([ONES, HALF], F32, tag="dsb")
                nc.vector.tensor_copy(dsb[:], pa[64:128, :])
                rd = rdp.tile([ONES, HALF], F32, tag="rd")
                nc.vector.reciprocal_approx_fast(rd[:], dsb[:])
                if dbg and h == 0 and half == 0:
                    nc.sync.dma_start(drd_d[:], rd[:])
                cols = slice(HALF * half, HALF * (half + 1))
                nc.vector.tensor_mul(aT[p][r0:r0 + 64, cols], pa[0:64, :], rd[:])

            # ---- prologue: K/Q slice 0 for head pair 0, V chunks 0-5 ----
            for n in range(2):
                k_chain(0, 0, n)
            for j in range(6):
                v_chunk(j)
            for n in range(2):
                q_chain(0, 0, n)

            # ---- attention with interleaved projections / output ----
            for half in range(2):
                for h in range(GH):
                    attn_block(h, half)
            for mo in range(D // 128):
                out_tail(mo)

            if dbg:
                for p in range(2):
                    nc.sync.dma_start(dq_d[:, LQ * p:LQ * (p + 1)], qT[p][:])
                    nc.sync.dma_start(dk_d[:, LQ * p:LQ * (p + 1)], kT[p][:])
                    nc.sync.dma_start(da_d[:, LQ * p:LQ * (p + 1)], aT[p][:])
                nc.sync.dma_start(dv_d[:], v_t[:])

    nc.compile()
    return nc


_NC_CACHE = []


def _get_nc():
    if not _NC_CACHE:
        _NC_CACHE.append(_build())
    return _NC_CACHE[0]


def kernel_run(inputs, trace=False, **kw):
    """Run on HW; returns (full_output, BassKernelResults)."""
    x = np.asarray(inputs["x"], np.float32)
    context = np.asarray(inputs["context"], np.float32)
    w_q = np.asarray(inputs["w_q"], np.float32)
    b_q = np.asarray(inputs["b_q"], np.float32)
    w_k = np.asarray(inputs["w_k"], np.float32)
    b_k = np.asarray(inputs["b_k"], np.float32)
    w_v = np.asarray(inputs["w_v"], np.float32)
    b_v = np.asarray(inputs["b_v"], np.float32)
    w_o = np.asarray(inputs["w_o"], np.float32)
    b_o = np.asarray(inputs["b_o"], np.float32)

    f16 = np.float16
    xT = [np.ascontiguousarray(x[b].T).astype(f16) for b in range(B)]
    cT = [np.ascontiguousarray(context[b].T).astype(f16) for b in range(B)]

    maps = []
    for c in range(8):
        b, g = c // 4, c % 4
        hs = slice(256 * g, 256 * (g + 1))
        maps.append({
            "xT": xT[b],
            "ctxT": cT[b],
            "wq": np.ascontiguousarray(w_q[:, hs]).astype(f16),
            "wk": np.ascontiguousarray(w_k[:, hs]).astype(f16),
            "wv": np.ascontiguousarray(w_v[:, hs]).astype(f16),
            "wo": np.ascontiguousarray(w_o[hs, :]).astype(f16),
            "bq": np.ascontiguousarray(b_q[hs].reshape(2, 128).T.astype(np.float32)),
            "bk": np.ascontiguousarray(b_k[hs].reshape(2, 128).T.astype(np.float32)),
            "bvb": np.broadcast_to(b_v[hs].astype(np.float32), (128, 256)).copy(),
        })

    nc = _get_nc()
    res = bass_utils.run_bass_kernel_spmd(nc, maps, core_ids=list(range(8)),
                                          trace=trace, **kw)
    out = np.empty((B, LQ, D), np.float32)
    for b in range(B):
        acc = res.results[4 * b]["outT"].astype(np.float32)
        for g in range(1, 4):
            acc = acc + res.results[4 * b + g]["outT"]
        out[b] = acc.T + b_o[None, :]
    return out, res


def kernel(**inputs) -> np.ndarray:
    out, _ = kernel_run(inputs)
    return out
